# revision 3
# baseline (speedup 1.0000x reference)
"""Trainium2 Bass kernel for nn_Actor (GRU-over-vehicles + MLP head), v2.

Data parallel: B=16384 split across 8 cores (2048 rows each), params replicated.

v2 layout highlights vs v1:
- x tiles carry a ones-row per 32-partition strip; ALL biases (b_r, b_z,
  b_ihn, b_hhn) are folded into the K=16 x-side matmuls, so no ACT biases or
  STT bias ops are needed in the GRU loop.
- The 8 x-side matmuls per (t,g) are K=16 row-tiled to strips 0/32/64/96 and
  issue concurrently (tile_position), costing ~2 matmul slots instead of 8.
- PSUM is organized as two [128, 2048] 4-bank tiles per (t,g):
    A = [r|m0  r|m1  z|m0  z|m1]   B = [xn|m0  xn|m1  hn|m0  hn|m1]
  so sigmoid runs as two [128,1024] ACTs and all DVE ops run at FD=1024.
- GRU elementwise: u = hn*r (TT), w = u+xn (TT), n = tanh(w), d = h-n
  (GpSimd), a = z*d, h' = n+a. h stored as one [128, 1024] tile per group
  (m-chunks side by side on the free axis).
- MLP ReLU is one [128, 2048] ACT per (layer, out-chunk) across all 4 batch
  groups, bias as per-partition AP.
"""

import numpy as np
import ml_dtypes

import concourse.bass as bass
import concourse.tile as tile
from concourse import bacc
from concourse import mybir
from concourse.bass_utils import run_bass_kernel_spmd

BF16 = mybir.dt.bfloat16
F32 = mybir.dt.float32
Act = mybir.ActivationFunctionType
Alu = mybir.AluOpType

B, V, F, H = 16384, 20, 15, 256
NCORES = 8
BL = B // NCORES          # 2048 batch rows per core
GN = 512                  # batch-group width (PSUM bank = 512 fp32)
NG = BL // GN             # 4 groups

_NC_CACHE = {}


def _build_nc():
    nc = bacc.Bacc("TRN2", target_bir_lowering=False, debug=False)

    xT_d = nc.dram_tensor("xT", [V, 128, BL], BF16, kind="ExternalInput")
    wih_d = nc.dram_tensor("wih_all", [128, 1024], BF16, kind="ExternalInput")
    whh_d = nc.dram_tensor("w_hhT", [2, 128, 768], BF16, kind="ExternalInput")
    w1x_d = nc.dram_tensor("w1T_x", [16, 1024], BF16, kind="ExternalInput")
    w1h_d = nc.dram_tensor("w1T_h", [2, 128, 1024], BF16, kind="ExternalInput")
    w2_d = nc.dram_tensor("w2T", [8, 128, 1024], BF16, kind="ExternalInput")
    w3_d = nc.dram_tensor("w3T", [8, 128, 512], BF16, kind="ExternalInput")
    w4_d = nc.dram_tensor("w4T", [4, 128, 256], BF16, kind="ExternalInput")
    wp_d = nc.dram_tensor("wpT", [2, 128, 1], BF16, kind="ExternalInput")
    b2_d = nc.dram_tensor("b2", [128, 8], F32, kind="ExternalInput")
    b3_d = nc.dram_tensor("b3", [128, 4], F32, kind="ExternalInput")
    b4_d = nc.dram_tensor("b4", [128, 2], F32, kind="ExternalInput")
    bp_d = nc.dram_tensor("bp", [1, 1], F32, kind="ExternalInput")
    out_d = nc.dram_tensor("out", [1, BL], F32, kind="ExternalOutput")

    with tile.TileContext(nc) as tc:
        with (
            tc.tile_pool(name="const", bufs=1) as consts,
            tc.tile_pool(name="psum", bufs=2, space=bass.MemorySpace.PSUM) as psum,
            tc.tile_pool(name="work", bufs=3) as work,
            tc.tile_pool(name="mlp", bufs=8) as mlp,
        ):
            def load(dram_ap, shape, dtype, tag):
                t = consts.tile(shape, dtype, tag=tag, name=tag)
                nc.sync.dma_start(t[:], dram_ap)
                return t

            wih = load(wih_d[:], [128, 1024], BF16, "wih")
            x0 = load(xT_d[0], [128, BL], BF16, "x0")  # persistent: MLP input
            whh = [load(whh_d[i], [128, 768], BF16, f"whh{i}") for i in range(2)]
            xtiles = {0: x0}

            def xtile(t):
                if t not in xtiles:
                    xt_ = work.tile([128, BL], BF16, tag="xt", name="xt", bufs=3)
                    nc.sync.dma_start(xt_[:], xT_d[t])
                    xtiles[t] = xt_
                return xtiles[t]

            w1x = load(w1x_d[:], [16, 1024], BF16, "w1x")
            w1h = [load(w1h_d[i], [128, 1024], BF16, f"w1h{i}") for i in range(2)]
            w2 = [load(w2_d[i], [128, 1024], BF16, f"w2_{i}") for i in range(8)]
            w3 = [load(w3_d[i], [128, 512], BF16, f"w3_{i}") for i in range(8)]
            w4 = [load(w4_d[i], [128, 256], BF16, f"w4_{i}") for i in range(4)]
            wp = [load(wp_d[i], [128, 1], BF16, f"wp{i}") for i in range(2)]
            b2 = load(b2_d[:], [128, 8], F32, "b2")
            b3 = load(b3_d[:], [128, 4], F32, "b3")
            b4 = load(b4_d[:], [128, 2], F32, "b4")
            bp = load(bp_d[:], [1, 1], F32, "bp")

            oT = consts.tile([1, BL], F32, tag="oT", name="oT")

            mm = nc.tensor.matmul
            h_cur = [None] * NG  # [128, 1024] bf16 per group: (m0 cols | m1 cols)

            # ---------------- GRU over V=20 vehicle steps ----------------
            for t in range(V):
                xa = xtiles[t] if t in xtiles else xtile(t)
                xtile(min(t + 2, V - 1))  # prefetch
                for g in range(NG):
                    gs = slice(GN * g, GN * (g + 1))

                    # A: [r m0 | r m1 | z m0 | z m1], B: [xn m0|xn m1|hn m0|hn m1]
                    pA = psum.tile([128, 2048], F32, tag="p2", name="pA")
                    pB = psum.tile([128, 2048], F32, tag="p2", name="pB")

                    # x-side: 8 K=16 matmuls row-tiled over strips 0/32/64/96.
                    # Biases ride on the ones-row (row 15 of each strip).
                    for c in range(4):  # rz chunks -> pA
                        s = 32 * c
                        mm(pA[:, 512 * c : 512 * (c + 1)],
                           wih[s : s + 16, 128 * c : 128 * (c + 1)],
                           xa[s : s + 16, gs],
                           start=True, stop=(t == 0),
                           tile_position=(s, 0))
                    for c in range(4):  # xn m0, xn m1, b_hhn m0, b_hhn m1 -> pB
                        s = 32 * c
                        mm(pB[:, 512 * c : 512 * (c + 1)],
                           wih[s : s + 16, 128 * (4 + c) : 128 * (5 + c)],
                           xa[s : s + 16, gs],
                           start=True, stop=(t == 0 or c < 2),
                           tile_position=(s, 0))

                    # h-side (t>0): rz into pA, hn into pB[:,1024:]
                    if t > 0:
                        hg = h_cur[g]
                        for c in range(4):
                            for k in range(2):
                                mm(pA[:, 512 * c : 512 * (c + 1)],
                                   whh[k][:, 128 * c : 128 * (c + 1)],
                                   hg[:, 512 * k : 512 * (k + 1)],
                                   start=False, stop=(k == 1))
                        for m in range(2):
                            for k in range(2):
                                mm(pB[:, 1024 + 512 * m : 1024 + 512 * (m + 1)],
                                   whh[k][:, 512 + 128 * m : 512 + 128 * (m + 1)],
                                   hg[:, 512 * k : 512 * (k + 1)],
                                   start=False, stop=(k == 1))

                    rz = work.tile([128, 2048], BF16, tag="rz", name="rz", bufs=2)
                    nc.scalar.activation(rz[:, 0:1024], pA[:, 0:1024], Act.Sigmoid)
                    nc.scalar.activation(rz[:, 1024:2048], pA[:, 1024:2048], Act.Sigmoid)

                    u_ = work.tile([128, 1024], BF16, tag="u", name="u", bufs=2)
                    nc.vector.tensor_mul(u_[:], pB[:, 1024:2048], rz[:, 0:1024])
                    w_ = work.tile([128, 1024], BF16, tag="w", name="w", bufs=2)
                    nc.vector.tensor_add(w_[:], u_[:], pB[:, 0:1024])
                    n_ = work.tile([128, 1024], BF16, tag="n", name="n", bufs=2)
                    nc.scalar.activation(n_[:], w_[:], Act.Tanh)

                    h_new = work.tile([128, 1024], BF16, tag=f"h{g}",
                                      name=f"h{g}", bufs=2)
                    a_ = work.tile([128, 1024], BF16, tag="a", name="a", bufs=2)
                    if t == 0:
                        nc.vector.tensor_mul(a_[:], rz[:, 1024:2048], n_[:])
                        nc.vector.tensor_sub(h_new[:], n_[:], a_[:])
                    else:
                        d_ = work.tile([128, 1024], BF16, tag="d", name="d", bufs=2)
                        nc.gpsimd.tensor_sub(d_[:], h_cur[g][:], n_[:])
                        nc.vector.tensor_mul(a_[:], rz[:, 1024:2048], d_[:])
                        nc.vector.tensor_add(h_new[:], n_[:], a_[:])
                    h_cur[g] = h_new

            # ---------------- MLP head ----------------
            GSL = [slice(GN * g, GN * (g + 1)) for g in range(NG)]

            def mlp_layer(n_out_chunks, k_tiles, rhs_of, w_of, bias_of, out_tag,
                          out_bufs):
                outs = []
                for m_ in range(n_out_chunks):
                    pq = psum.tile([128, 2048], F32, tag="p2", name="pq")
                    for ki in range(k_tiles):
                        for g in range(NG):
                            mm(pq[:, 512 * g : 512 * (g + 1)], w_of(ki, m_),
                               rhs_of(ki, g),
                               start=(ki == 0), stop=(ki == k_tiles - 1))
                    a_t = mlp.tile([128, BL], BF16, tag=out_tag, name=out_tag,
                                   bufs=out_bufs)
                    bias = bias_of(m_)
                    if bias is None:
                        nc.scalar.activation(a_t[:], pq[:], Act.Relu)
                    else:
                        nc.scalar.activation(a_t[:], pq[:], Act.Relu, bias=bias)
                    outs.append(a_t)
                return outs

            def l1_rhs(ki, g):
                if ki == 0:
                    return x0[0:16, GSL[g]]
                return h_cur[ki - 1][:, 512 * (ki - 1) : 512 * ki]

            # layer-1 h rhs: k-chunk k comes from h_cur[g] columns, per group
            def l1_rhs2(ki, g):
                if ki == 0:
                    return x0[0:16, GSL[g]]
                k = ki - 1
                return h_cur[g][:, 512 * k : 512 * (k + 1)]

            def l1_w(ki, m_):
                if ki == 0:
                    return w1x[:, 128 * m_ : 128 * (m_ + 1)]
                return w1h[ki - 1][:, 128 * m_ : 128 * (m_ + 1)]

            a1 = mlp_layer(8, 3, l1_rhs2, l1_w, lambda m_: None, "a1", 8)
            a2 = mlp_layer(8, 8, lambda ki, g: a1[ki][:, GSL[g]],
                           lambda ki, m_: w2[ki][:, 128 * m_ : 128 * (m_ + 1)],
                           lambda m_: b2[:, m_ : m_ + 1], "a2", 8)
            a3 = mlp_layer(4, 8, lambda ki, g: a2[ki][:, GSL[g]],
                           lambda ki, m_: w3[ki][:, 128 * m_ : 128 * (m_ + 1)],
                           lambda m_: b3[:, m_ : m_ + 1], "a3", 4)
            a4 = mlp_layer(2, 4, lambda ki, g: a3[ki][:, GSL[g]],
                           lambda ki, m_: w4[ki][:, 128 * m_ : 128 * (m_ + 1)],
                           lambda m_: b4[:, m_ : m_ + 1], "a4", 2)

            po = psum.tile([1, 2048], F32, tag="p2", name="po")
            for ki in range(2):
                for g in range(NG):
                    mm(po[0:1, GSL[g]], wp[ki][:, 0:1], a4[ki][:, GSL[g]],
                       start=(ki == 0), stop=(ki == 1))
            nc.scalar.activation(oT[0:1, :], po[0:1, :], Act.Tanh,
                                 bias=bp[0:1, 0:1])

            nc.sync.dma_start(out_d[:], oT[:])

    nc.compile()
    return nc


def _get_nc():
    if "nc" not in _NC_CACHE:
        _NC_CACHE["nc"] = _build_nc()
    return _NC_CACHE["nc"]


def _prep_shared(inputs):
    f4 = np.float32
    bf = ml_dtypes.bfloat16

    def g(name):
        return np.asarray(inputs[name], dtype=f4)

    W_ih, W_hh = g("W_ih"), g("W_hh")
    b_ih, b_hh = g("b_ih"), g("b_hh")
    W1, W2, W3, W4, Wp = g("W1"), g("W2"), g("W3"), g("W4"), g("Wp")
    b1, b2, b3, b4, bp = g("b1"), g("b2"), g("b3"), g("b4"), g("bp")

    # wih_all [128, 1024]: 8 column blocks; block c uses partition strip c%4.
    # blocks 0-3: rz gate chunks (bias = b_ih+b_hh); 4-5: xn chunks
    # (bias = b_ih n-part); 6-7: zero weights + b_hh n-part bias only.
    wih_all = np.zeros((128, 1024), dtype=f4)
    bsum = b_ih + b_hh
    for c in range(4):
        s = 32 * c
        wih_all[s : s + 15, 128 * c : 128 * (c + 1)] = W_ih[128 * c : 128 * (c + 1)].T
        wih_all[s + 15, 128 * c : 128 * (c + 1)] = bsum[128 * c : 128 * (c + 1)]
    for m in range(2):
        c, s = 4 + m, 32 * m
        lo = 512 + 128 * m
        wih_all[s : s + 15, 128 * c : 128 * (c + 1)] = W_ih[lo : lo + 128].T
        wih_all[s + 15, 128 * c : 128 * (c + 1)] = b_ih[lo : lo + 128]
    for m in range(2):
        c, s = 6 + m, 32 * (2 + m)
        lo = 512 + 128 * m
        wih_all[s + 15, 128 * c : 128 * (c + 1)] = b_hh[lo : lo + 128]

    w1x = np.zeros((16, 1024), dtype=f4)
    w1x[0:15] = W1.T[0:15]
    w1x[15] = b1

    shared = {
        "wih_all": wih_all.astype(bf),
        "w_hhT": np.ascontiguousarray(W_hh.T.reshape(2, 128, 768)).astype(bf),
        "w1T_x": w1x.astype(bf),
        "w1T_h": np.ascontiguousarray(W1.T[15:].reshape(2, 128, 1024)).astype(bf),
        "w2T": np.ascontiguousarray(W2.T.reshape(8, 128, 1024)).astype(bf),
        "w3T": np.ascontiguousarray(W3.T.reshape(8, 128, 512)).astype(bf),
        "w4T": np.ascontiguousarray(W4.T.reshape(4, 128, 256)).astype(bf),
        "wpT": np.ascontiguousarray(Wp.T.reshape(2, 128, 1)).astype(bf),
        "b2": np.ascontiguousarray(b2.reshape(8, 128).T),
        "b3": np.ascontiguousarray(b3.reshape(4, 128).T),
        "b4": np.ascontiguousarray(b4.reshape(2, 128).T),
        "bp": bp.reshape(1, 1).astype(f4),
    }
    return shared


def _prep_xT(state_core):
    """state [BL, 20, 15] f32 -> [20, 128, BL] bf16: features replicated at
    partition strips 0/32/64/96; row 15 of each strip is the all-ones bias
    row; rows 16-31 zero."""
    s = state_core.transpose(1, 2, 0)  # [20, 15, BL]
    xp = np.zeros((V, 4, 32, BL), dtype=np.float32)
    xp[:, :, :F, :] = s[:, None, :, :]
    xp[:, :, 15, :] = 1.0
    return xp.reshape(V, 128, BL).astype(ml_dtypes.bfloat16)


def run(inputs, trace=False):
    nc = _get_nc()
    shared = _prep_shared(inputs)
    state = np.asarray(inputs["state"], dtype=np.float32)
    in_maps = []
    for c in range(NCORES):
        m = dict(shared)
        m["xT"] = _prep_xT(state[BL * c : BL * (c + 1)])
        in_maps.append(m)
    res = run_bass_kernel_spmd(nc, in_maps, list(range(NCORES)), trace=trace)
    out = np.concatenate(
        [np.asarray(res.results[c]["out"]).reshape(BL) for c in range(NCORES)]
    )
    return out.reshape(B, 1).astype(np.float32), res


def kernel(**inputs):
    out, _ = run(inputs, trace=False)
    return out


# revision 4
# speedup vs baseline: 1.2817x; 1.2817x over previous
"""Trainium2 Bass kernel for nn_Actor (GRU-over-vehicles + MLP head), v2.

Data parallel: B=16384 split across 8 cores (2048 rows each), params replicated.

v2 layout highlights vs v1:
- x tiles carry a ones-row per 32-partition strip; ALL biases (b_r, b_z,
  b_ihn, b_hhn) are folded into the K=16 x-side matmuls, so no ACT biases or
  STT bias ops are needed in the GRU loop.
- The 8 x-side matmuls per (t,g) are K=16 row-tiled to strips 0/32/64/96 and
  issue concurrently (tile_position), costing ~2 matmul slots instead of 8.
- PSUM is organized as two [128, 2048] 4-bank tiles per (t,g):
    A = [r|m0  r|m1  z|m0  z|m1]   B = [xn|m0  xn|m1  hn|m0  hn|m1]
  so sigmoid runs as two [128,1024] ACTs and all DVE ops run at FD=1024.
- GRU elementwise: u = hn*r (TT), w = u+xn (TT), n = tanh(w), d = h-n
  (GpSimd), a = z*d, h' = n+a. h stored as one [128, 1024] tile per group
  (m-chunks side by side on the free axis).
- MLP ReLU is one [128, 2048] ACT per (layer, out-chunk) across all 4 batch
  groups, bias as per-partition AP.
"""

import numpy as np
import ml_dtypes

import concourse.bass as bass
import concourse.tile as tile
from concourse import bacc
from concourse import mybir
from concourse.bass_utils import run_bass_kernel_spmd

BF16 = mybir.dt.bfloat16
F32 = mybir.dt.float32
Act = mybir.ActivationFunctionType
Alu = mybir.AluOpType

B, V, F, H = 16384, 20, 15, 256
NCORES = 8
BL = B // NCORES          # 2048 batch rows per core
GN = 512                  # batch-group width (PSUM bank = 512 fp32)
NG = BL // GN             # 4 groups

_NC_CACHE = {}


def _build_nc():
    nc = bacc.Bacc("TRN2", target_bir_lowering=False, debug=False)

    xT_d = nc.dram_tensor("xT", [V, 128, BL], BF16, kind="ExternalInput")
    wih_d = nc.dram_tensor("wih_all", [128, 1024], BF16, kind="ExternalInput")
    whh_d = nc.dram_tensor("w_hhT", [2, 128, 768], BF16, kind="ExternalInput")
    w1x_d = nc.dram_tensor("w1T_x", [16, 1024], BF16, kind="ExternalInput")
    w1h_d = nc.dram_tensor("w1T_h", [2, 128, 1024], BF16, kind="ExternalInput")
    w2_d = nc.dram_tensor("w2T", [8, 128, 1024], BF16, kind="ExternalInput")
    w3_d = nc.dram_tensor("w3T", [8, 128, 512], BF16, kind="ExternalInput")
    w4_d = nc.dram_tensor("w4T", [4, 128, 256], BF16, kind="ExternalInput")
    wp_d = nc.dram_tensor("wpT", [2, 128, 1], BF16, kind="ExternalInput")
    b2_d = nc.dram_tensor("b2", [128, 8], F32, kind="ExternalInput")
    b3_d = nc.dram_tensor("b3", [128, 4], F32, kind="ExternalInput")
    b4_d = nc.dram_tensor("b4", [128, 2], F32, kind="ExternalInput")
    bp_d = nc.dram_tensor("bp", [1, 1], F32, kind="ExternalInput")
    out_d = nc.dram_tensor("out", [1, BL], F32, kind="ExternalOutput")

    with tile.TileContext(nc) as tc:
        with (
            tc.tile_pool(name="const", bufs=1) as consts,
            tc.tile_pool(name="psum", bufs=2, space=bass.MemorySpace.PSUM) as psum,
            tc.tile_pool(name="work", bufs=3) as work,
            tc.tile_pool(name="mlp", bufs=8) as mlp,
        ):
            def load(dram_ap, shape, dtype, tag):
                t = consts.tile(shape, dtype, tag=tag, name=tag)
                nc.sync.dma_start(t[:], dram_ap)
                return t

            wih = load(wih_d[:], [128, 1024], BF16, "wih")
            x0 = load(xT_d[0], [128, BL], BF16, "x0")  # persistent: MLP input
            whh = [load(whh_d[i], [128, 768], BF16, f"whh{i}") for i in range(2)]
            xtiles = {0: x0}

            def xtile(t):
                if t not in xtiles:
                    xt_ = work.tile([128, BL], BF16, tag="xt", name="xt", bufs=3)
                    nc.sync.dma_start(xt_[:], xT_d[t])
                    xtiles[t] = xt_
                return xtiles[t]

            w1x = load(w1x_d[:], [16, 1024], BF16, "w1x")
            w1h = [load(w1h_d[i], [128, 1024], BF16, f"w1h{i}") for i in range(2)]
            w2 = [load(w2_d[i], [128, 1024], BF16, f"w2_{i}") for i in range(8)]
            w3 = [load(w3_d[i], [128, 512], BF16, f"w3_{i}") for i in range(8)]
            w4 = [load(w4_d[i], [128, 256], BF16, f"w4_{i}") for i in range(4)]
            wp = [load(wp_d[i], [128, 1], BF16, f"wp{i}") for i in range(2)]
            b2 = load(b2_d[:], [128, 8], F32, "b2")
            b3 = load(b3_d[:], [128, 4], F32, "b3")
            b4 = load(b4_d[:], [128, 2], F32, "b4")
            bp = load(bp_d[:], [1, 1], F32, "bp")

            oT = consts.tile([1, BL], F32, tag="oT", name="oT")

            mm = nc.tensor.matmul
            h_cur = [None] * NG  # [128, 1024] bf16 per group: (m0 cols | m1 cols)

            # ---------------- GRU over V=20 vehicle steps ----------------
            for t in range(V):
                xa = xtiles[t] if t in xtiles else xtile(t)
                xtile(min(t + 2, V - 1))  # prefetch
                for g in range(NG):
                    gs = slice(GN * g, GN * (g + 1))

                    # 2-bank psum tiles: finer free granularity for pipelining
                    pR = psum.tile([128, 1024], F32, tag="pR", name="pR", bufs=1)
                    pZ = psum.tile([128, 1024], F32, tag="pZ", name="pZ", bufs=1)
                    pXN = psum.tile([128, 1024], F32, tag="pXN", name="pXN", bufs=1)
                    pHN = psum.tile([128, 1024], F32, tag="pHN", name="pHN", bufs=1)

                    # x-side: 8 K=16 matmuls row-tiled over strips 0/32/64/96.
                    # Biases ride on the ones-row (row 15 of each strip).
                    rz_dst = [pR[:, 0:512], pR[:, 512:1024],
                              pZ[:, 0:512], pZ[:, 512:1024]]
                    xb_dst = [pXN[:, 0:512], pXN[:, 512:1024],
                              pHN[:, 0:512], pHN[:, 512:1024]]
                    for c in range(4):  # r0 r1 z0 z1
                        s = 32 * c
                        mm(rz_dst[c],
                           wih[s : s + 16, 128 * c : 128 * (c + 1)],
                           xa[s : s + 16, gs],
                           start=True, stop=(t == 0),
                           tile_position=(s, 0))
                    for c in range(4):  # xn m0, xn m1, b_hhn m0, b_hhn m1
                        s = 32 * c
                        mm(xb_dst[c],
                           wih[s : s + 16, 128 * (4 + c) : 128 * (5 + c)],
                           xa[s : s + 16, gs],
                           start=True, stop=(t == 0 or c < 2),
                           tile_position=(s, 0))

                    # h-side (t>0)
                    if t > 0:
                        hg = h_cur[g]
                        for c in range(4):
                            for k in range(2):
                                mm(rz_dst[c],
                                   whh[k][:, 128 * c : 128 * (c + 1)],
                                   hg[:, 512 * k : 512 * (k + 1)],
                                   start=False, stop=(k == 1))
                        for m in range(2):
                            for k in range(2):
                                mm(xb_dst[2 + m],
                                   whh[k][:, 512 + 128 * m : 512 + 128 * (m + 1)],
                                   hg[:, 512 * k : 512 * (k + 1)],
                                   start=False, stop=(k == 1))

                    rz = work.tile([128, 2048], BF16, tag="rz", name="rz", bufs=2)
                    nc.scalar.activation(rz[:, 0:1024], pR[:], Act.Sigmoid)
                    nc.scalar.activation(rz[:, 1024:2048], pZ[:], Act.Sigmoid)

                    u_ = work.tile([128, 1024], BF16, tag="u", name="u", bufs=2)
                    nc.vector.tensor_mul(u_[:], pHN[:], rz[:, 0:1024])
                    w_ = work.tile([128, 1024], BF16, tag="w", name="w", bufs=2)
                    nc.vector.tensor_add(w_[:], u_[:], pXN[:])
                    n_ = work.tile([128, 1024], BF16, tag="n", name="n", bufs=2)
                    nc.scalar.activation(n_[:], w_[:], Act.Tanh)

                    h_new = work.tile([128, 1024], BF16, tag=f"h{g}",
                                      name=f"h{g}", bufs=2)
                    a_ = work.tile([128, 1024], BF16, tag="a", name="a", bufs=2)
                    if t == 0:
                        nc.vector.tensor_mul(a_[:], rz[:, 1024:2048], n_[:])
                        nc.vector.tensor_sub(h_new[:], n_[:], a_[:])
                    else:
                        d_ = work.tile([128, 1024], BF16, tag="d", name="d", bufs=2)
                        nc.gpsimd.tensor_sub(d_[:], h_cur[g][:], n_[:])
                        if g % 2 == 0:
                            nc.vector.tensor_mul(a_[:], rz[:, 1024:2048], d_[:])
                        else:
                            nc.gpsimd.tensor_mul(a_[:], rz[:, 1024:2048], d_[:])
                        nc.vector.tensor_add(h_new[:], n_[:], a_[:])
                    h_cur[g] = h_new

            # ---------------- MLP head ----------------
            GSL = [slice(GN * g, GN * (g + 1)) for g in range(NG)]

            def mlp_layer(n_out_chunks, k_tiles, rhs_of, w_of, bias_of, out_tag,
                          out_bufs):
                outs = []
                tagp = [("pR", "pZ"), ("pXN", "pHN")]
                for m_ in range(n_out_chunks):
                    tg = tagp[m_ % 2]
                    pq0 = psum.tile([128, 1024], F32, tag=tg[0], name="pq0", bufs=1)
                    pq1 = psum.tile([128, 1024], F32, tag=tg[1], name="pq1", bufs=1)
                    halves = [pq0[:, 0:512], pq0[:, 512:1024],
                              pq1[:, 0:512], pq1[:, 512:1024]]
                    for ki in range(k_tiles):
                        for g in range(NG):
                            mm(halves[g], w_of(ki, m_), rhs_of(ki, g),
                               start=(ki == 0), stop=(ki == k_tiles - 1))
                    a_t = mlp.tile([128, BL], BF16, tag=out_tag, name=out_tag,
                                   bufs=out_bufs)
                    bias = bias_of(m_)
                    if bias is None:
                        nc.scalar.activation(a_t[:, 0:1024], pq0[:], Act.Relu)
                        nc.scalar.activation(a_t[:, 1024:2048], pq1[:], Act.Relu)
                    else:
                        nc.scalar.activation(a_t[:, 0:1024], pq0[:], Act.Relu,
                                             bias=bias)
                        nc.scalar.activation(a_t[:, 1024:2048], pq1[:], Act.Relu,
                                             bias=bias)
                    outs.append(a_t)
                return outs

            def l1_rhs(ki, g):
                if ki == 0:
                    return x0[0:16, GSL[g]]
                return h_cur[ki - 1][:, 512 * (ki - 1) : 512 * ki]

            # layer-1 h rhs: k-chunk k comes from h_cur[g] columns, per group
            def l1_rhs2(ki, g):
                if ki == 0:
                    return x0[0:16, GSL[g]]
                k = ki - 1
                return h_cur[g][:, 512 * k : 512 * (k + 1)]

            def l1_w(ki, m_):
                if ki == 0:
                    return w1x[:, 128 * m_ : 128 * (m_ + 1)]
                return w1h[ki - 1][:, 128 * m_ : 128 * (m_ + 1)]

            a1 = mlp_layer(8, 3, l1_rhs2, l1_w, lambda m_: None, "a1", 8)
            a2 = mlp_layer(8, 8, lambda ki, g: a1[ki][:, GSL[g]],
                           lambda ki, m_: w2[ki][:, 128 * m_ : 128 * (m_ + 1)],
                           lambda m_: b2[:, m_ : m_ + 1], "a2", 8)
            a3 = mlp_layer(4, 8, lambda ki, g: a2[ki][:, GSL[g]],
                           lambda ki, m_: w3[ki][:, 128 * m_ : 128 * (m_ + 1)],
                           lambda m_: b3[:, m_ : m_ + 1], "a3", 4)
            a4 = mlp_layer(2, 4, lambda ki, g: a3[ki][:, GSL[g]],
                           lambda ki, m_: w4[ki][:, 128 * m_ : 128 * (m_ + 1)],
                           lambda m_: b4[:, m_ : m_ + 1], "a4", 2)

            po0 = psum.tile([1, 1024], F32, tag="pR", name="po0", bufs=1)
            po1 = psum.tile([1, 1024], F32, tag="pZ", name="po1", bufs=1)
            phalf = [po0[0:1, 0:512], po0[0:1, 512:1024],
                     po1[0:1, 0:512], po1[0:1, 512:1024]]
            for ki in range(2):
                for g in range(NG):
                    mm(phalf[g], wp[ki][:, 0:1], a4[ki][:, GSL[g]],
                       start=(ki == 0), stop=(ki == 1))
            nc.scalar.activation(oT[0:1, 0:1024], po0[0:1, :], Act.Tanh,
                                 bias=bp[0:1, 0:1])
            nc.scalar.activation(oT[0:1, 1024:2048], po1[0:1, :], Act.Tanh,
                                 bias=bp[0:1, 0:1])

            nc.sync.dma_start(out_d[:], oT[:])

    nc.compile()
    return nc


def _get_nc():
    if "nc" not in _NC_CACHE:
        _NC_CACHE["nc"] = _build_nc()
    return _NC_CACHE["nc"]


def _prep_shared(inputs):
    f4 = np.float32
    bf = ml_dtypes.bfloat16

    def g(name):
        return np.asarray(inputs[name], dtype=f4)

    W_ih, W_hh = g("W_ih"), g("W_hh")
    b_ih, b_hh = g("b_ih"), g("b_hh")
    W1, W2, W3, W4, Wp = g("W1"), g("W2"), g("W3"), g("W4"), g("Wp")
    b1, b2, b3, b4, bp = g("b1"), g("b2"), g("b3"), g("b4"), g("bp")

    # wih_all [128, 1024]: 8 column blocks; block c uses partition strip c%4.
    # blocks 0-3: rz gate chunks (bias = b_ih+b_hh); 4-5: xn chunks
    # (bias = b_ih n-part); 6-7: zero weights + b_hh n-part bias only.
    wih_all = np.zeros((128, 1024), dtype=f4)
    bsum = b_ih + b_hh
    for c in range(4):
        s = 32 * c
        wih_all[s : s + 15, 128 * c : 128 * (c + 1)] = W_ih[128 * c : 128 * (c + 1)].T
        wih_all[s + 15, 128 * c : 128 * (c + 1)] = bsum[128 * c : 128 * (c + 1)]
    for m in range(2):
        c, s = 4 + m, 32 * m
        lo = 512 + 128 * m
        wih_all[s : s + 15, 128 * c : 128 * (c + 1)] = W_ih[lo : lo + 128].T
        wih_all[s + 15, 128 * c : 128 * (c + 1)] = b_ih[lo : lo + 128]
    for m in range(2):
        c, s = 6 + m, 32 * (2 + m)
        lo = 512 + 128 * m
        wih_all[s + 15, 128 * c : 128 * (c + 1)] = b_hh[lo : lo + 128]

    w1x = np.zeros((16, 1024), dtype=f4)
    w1x[0:15] = W1.T[0:15]
    w1x[15] = b1

    shared = {
        "wih_all": wih_all.astype(bf),
        "w_hhT": np.ascontiguousarray(W_hh.T.reshape(2, 128, 768)).astype(bf),
        "w1T_x": w1x.astype(bf),
        "w1T_h": np.ascontiguousarray(W1.T[15:].reshape(2, 128, 1024)).astype(bf),
        "w2T": np.ascontiguousarray(W2.T.reshape(8, 128, 1024)).astype(bf),
        "w3T": np.ascontiguousarray(W3.T.reshape(8, 128, 512)).astype(bf),
        "w4T": np.ascontiguousarray(W4.T.reshape(4, 128, 256)).astype(bf),
        "wpT": np.ascontiguousarray(Wp.T.reshape(2, 128, 1)).astype(bf),
        "b2": np.ascontiguousarray(b2.reshape(8, 128).T),
        "b3": np.ascontiguousarray(b3.reshape(4, 128).T),
        "b4": np.ascontiguousarray(b4.reshape(2, 128).T),
        "bp": bp.reshape(1, 1).astype(f4),
    }
    return shared


def _prep_xT(state_core):
    """state [BL, 20, 15] f32 -> [20, 128, BL] bf16: features replicated at
    partition strips 0/32/64/96; row 15 of each strip is the all-ones bias
    row; rows 16-31 zero."""
    s = state_core.transpose(1, 2, 0)  # [20, 15, BL]
    xp = np.zeros((V, 4, 32, BL), dtype=np.float32)
    xp[:, :, :F, :] = s[:, None, :, :]
    xp[:, :, 15, :] = 1.0
    return xp.reshape(V, 128, BL).astype(ml_dtypes.bfloat16)


def run(inputs, trace=False):
    nc = _get_nc()
    shared = _prep_shared(inputs)
    state = np.asarray(inputs["state"], dtype=np.float32)
    in_maps = []
    for c in range(NCORES):
        m = dict(shared)
        m["xT"] = _prep_xT(state[BL * c : BL * (c + 1)])
        in_maps.append(m)
    res = run_bass_kernel_spmd(nc, in_maps, list(range(NCORES)), trace=trace)
    out = np.concatenate(
        [np.asarray(res.results[c]["out"]).reshape(BL) for c in range(NCORES)]
    )
    return out.reshape(B, 1).astype(np.float32), res


def kernel(**inputs):
    out, _ = run(inputs, trace=False)
    return out


# revision 5
# speedup vs baseline: 1.6518x; 1.2887x over previous
"""Trainium2 Bass kernel for nn_Actor (GRU-over-vehicles + MLP head), v2.

Data parallel: B=16384 split across 8 cores (2048 rows each), params replicated.

v2 layout highlights vs v1:
- x tiles carry a ones-row per 32-partition strip; ALL biases (b_r, b_z,
  b_ihn, b_hhn) are folded into the K=16 x-side matmuls, so no ACT biases or
  STT bias ops are needed in the GRU loop.
- The 8 x-side matmuls per (t,g) are K=16 row-tiled to strips 0/32/64/96 and
  issue concurrently (tile_position), costing ~2 matmul slots instead of 8.
- PSUM is organized as two [128, 2048] 4-bank tiles per (t,g):
    A = [r|m0  r|m1  z|m0  z|m1]   B = [xn|m0  xn|m1  hn|m0  hn|m1]
  so sigmoid runs as two [128,1024] ACTs and all DVE ops run at FD=1024.
- GRU elementwise: u = hn*r (TT), w = u+xn (TT), n = tanh(w), d = h-n
  (GpSimd), a = z*d, h' = n+a. h stored as one [128, 1024] tile per group
  (m-chunks side by side on the free axis).
- MLP ReLU is one [128, 2048] ACT per (layer, out-chunk) across all 4 batch
  groups, bias as per-partition AP.
"""

import numpy as np
import ml_dtypes

import concourse.bass as bass
import concourse.tile as tile
from concourse import bacc
from concourse import mybir
from concourse.bass_utils import run_bass_kernel_spmd

BF16 = mybir.dt.bfloat16
F32 = mybir.dt.float32
Act = mybir.ActivationFunctionType
Alu = mybir.AluOpType

B, V, F, H = 16384, 20, 15, 256
NCORES = 8
BL = B // NCORES          # 2048 batch rows per core
GN = 512                  # batch-group width (PSUM bank = 512 fp32)
NG = BL // GN             # 4 groups

_NC_CACHE = {}


def _build_nc():
    nc = bacc.Bacc("TRN2", target_bir_lowering=False, debug=False)

    xT_d = nc.dram_tensor("xT", [V, 128, BL], BF16, kind="ExternalInput")
    wih_d = nc.dram_tensor("wih_all", [128, 1024], BF16, kind="ExternalInput")
    whh_d = nc.dram_tensor("w_hhT", [2, 128, 768], BF16, kind="ExternalInput")
    w1x_d = nc.dram_tensor("w1T_x", [16, 1024], BF16, kind="ExternalInput")
    w1h_d = nc.dram_tensor("w1T_h", [2, 128, 1024], BF16, kind="ExternalInput")
    w2_d = nc.dram_tensor("w2T", [8, 128, 1024], BF16, kind="ExternalInput")
    w3_d = nc.dram_tensor("w3T", [8, 128, 512], BF16, kind="ExternalInput")
    w4_d = nc.dram_tensor("w4T", [4, 128, 256], BF16, kind="ExternalInput")
    wp_d = nc.dram_tensor("wpT", [2, 128, 1], BF16, kind="ExternalInput")
    b2_d = nc.dram_tensor("b2", [128, 8], F32, kind="ExternalInput")
    b3_d = nc.dram_tensor("b3", [128, 4], F32, kind="ExternalInput")
    b4_d = nc.dram_tensor("b4", [128, 2], F32, kind="ExternalInput")
    bp_d = nc.dram_tensor("bp", [1, 1], F32, kind="ExternalInput")
    out_d = nc.dram_tensor("out", [1, BL], F32, kind="ExternalOutput")

    with tile.TileContext(nc) as tc:
        with (
            tc.tile_pool(name="const", bufs=1) as consts,
            tc.tile_pool(name="psum", bufs=2, space=bass.MemorySpace.PSUM) as psum,
            tc.tile_pool(name="work", bufs=3) as work,
            tc.tile_pool(name="mlp", bufs=8) as mlp,
        ):
            def load(dram_ap, shape, dtype, tag):
                t = consts.tile(shape, dtype, tag=tag, name=tag)
                nc.sync.dma_start(t[:], dram_ap)
                return t

            wih = load(wih_d[:], [128, 1024], BF16, "wih")
            x0 = load(xT_d[0], [128, BL], BF16, "x0")  # persistent: MLP input
            whh = [load(whh_d[i], [128, 768], BF16, f"whh{i}") for i in range(2)]
            xtiles = {0: x0}

            def xtile(t):
                if t not in xtiles:
                    xt_ = work.tile([128, BL], BF16, tag="xt", name="xt", bufs=3)
                    nc.sync.dma_start(xt_[:], xT_d[t])
                    xtiles[t] = xt_
                return xtiles[t]

            w1x = load(w1x_d[:], [16, 1024], BF16, "w1x")
            w1h = [load(w1h_d[i], [128, 1024], BF16, f"w1h{i}") for i in range(2)]
            w2 = [load(w2_d[i], [128, 1024], BF16, f"w2_{i}") for i in range(8)]
            w3 = [load(w3_d[i], [128, 512], BF16, f"w3_{i}") for i in range(8)]
            w4 = [load(w4_d[i], [128, 256], BF16, f"w4_{i}") for i in range(4)]
            wp = [load(wp_d[i], [128, 1], BF16, f"wp{i}") for i in range(2)]
            b2 = load(b2_d[:], [128, 8], F32, "b2")
            b3 = load(b3_d[:], [128, 4], F32, "b3")
            b4 = load(b4_d[:], [128, 2], F32, "b4")
            bp = load(bp_d[:], [1, 1], F32, "bp")

            oT = consts.tile([1, BL], F32, tag="oT", name="oT")

            mm = nc.tensor.matmul
            h_cur = [None] * NG  # [128, 1024] bf16 per group: (m0 cols | m1 cols)

            # ---------------- GRU over V=20 vehicle steps ----------------
            for t in range(V):
                xa = xtiles[t] if t in xtiles else xtile(t)
                xtile(min(t + 2, V - 1))  # prefetch
                for g in range(NG):
                    gs = slice(GN * g, GN * (g + 1))

                    # 2-bank psum tiles: finer free granularity for pipelining
                    pR = psum.tile([128, 1024], F32, tag="pR", name="pR", bufs=1)
                    pZ = psum.tile([128, 1024], F32, tag="pZ", name="pZ", bufs=1)
                    pXN = psum.tile([128, 1024], F32, tag="pXN", name="pXN", bufs=1)
                    pHN = psum.tile([128, 1024], F32, tag="pHN", name="pHN", bufs=1)

                    # x-side: 8 K=16 matmuls row-tiled over strips 0/32/64/96.
                    # Biases ride on the ones-row (row 15 of each strip).
                    rz_dst = [pR[:, 0:512], pR[:, 512:1024],
                              pZ[:, 0:512], pZ[:, 512:1024]]
                    xb_dst = [pXN[:, 0:512], pXN[:, 512:1024],
                              pHN[:, 0:512], pHN[:, 512:1024]]
                    for c in range(4):  # r0 r1 z0 z1
                        s = 32 * c
                        mm(rz_dst[c],
                           wih[s : s + 16, 128 * c : 128 * (c + 1)],
                           xa[s : s + 16, gs],
                           start=True, stop=(t == 0),
                           tile_position=(s, 0))
                    for c in range(4):  # xn m0, xn m1, b_hhn m0, b_hhn m1
                        s = 32 * c
                        mm(xb_dst[c],
                           wih[s : s + 16, 128 * (4 + c) : 128 * (5 + c)],
                           xa[s : s + 16, gs],
                           start=True, stop=(t == 0 or c < 2),
                           tile_position=(s, 0))

                    # h-side (t>0)
                    if t > 0:
                        hg = h_cur[g]
                        for c in range(4):
                            for k in range(2):
                                mm(rz_dst[c],
                                   whh[k][:, 128 * c : 128 * (c + 1)],
                                   hg[:, 512 * k : 512 * (k + 1)],
                                   start=False, stop=(k == 1))
                        for m in range(2):
                            for k in range(2):
                                mm(xb_dst[2 + m],
                                   whh[k][:, 512 + 128 * m : 512 + 128 * (m + 1)],
                                   hg[:, 512 * k : 512 * (k + 1)],
                                   start=False, stop=(k == 1))

                    rz = work.tile([128, 2048], BF16, tag="rz", name="rz", bufs=2)
                    nc.scalar.activation(rz[:, 0:1024], pR[:], Act.Sigmoid)
                    nc.scalar.activation(rz[:, 1024:2048], pZ[:], Act.Sigmoid)

                    u_ = work.tile([128, 1024], BF16, tag="u", name="u", bufs=2)
                    nc.vector.tensor_mul(u_[:], pHN[:], rz[:, 0:1024])
                    w_ = work.tile([128, 1024], BF16, tag="w", name="w", bufs=2)
                    if g % 2 == 0:
                        # ScalarE evacuates xn (frees pXN early, offloads DVE)
                        xn_s = work.tile([128, 1024], BF16, tag="xs",
                                         name="xn_s", bufs=2)
                        nc.scalar.activation(xn_s[:], pXN[:], Act.Copy)
                        nc.vector.tensor_add(w_[:], u_[:], xn_s[:])
                    else:
                        nc.vector.tensor_add(w_[:], u_[:], pXN[:])
                    n_ = work.tile([128, 1024], BF16, tag="n", name="n", bufs=2)
                    nc.scalar.activation(n_[:], w_[:], Act.Tanh)

                    h_new = work.tile([128, 1024], BF16, tag=f"h{g}",
                                      name=f"h{g}", bufs=2)
                    a_ = work.tile([128, 1024], BF16, tag="a", name="a", bufs=2)
                    if t == 0:
                        nc.vector.tensor_mul(a_[:], rz[:, 1024:2048], n_[:])
                        nc.vector.tensor_sub(h_new[:], n_[:], a_[:])
                    else:
                        d_ = work.tile([128, 1024], BF16, tag="d", name="d", bufs=2)
                        nc.vector.tensor_sub(d_[:], h_cur[g][:], n_[:])
                        nc.vector.tensor_mul(a_[:], rz[:, 1024:2048], d_[:])
                        nc.vector.tensor_add(h_new[:], n_[:], a_[:])
                    h_cur[g] = h_new

            # ---------------- MLP head ----------------
            GSL = [slice(GN * g, GN * (g + 1)) for g in range(NG)]

            def mlp_layer(n_out_chunks, k_tiles, rhs_of, w_of, bias_of, out_tag,
                          out_bufs):
                outs = []
                tagp = [("pR", "pZ"), ("pXN", "pHN")]
                for m_ in range(n_out_chunks):
                    tg = tagp[m_ % 2]
                    pq0 = psum.tile([128, 1024], F32, tag=tg[0], name="pq0", bufs=1)
                    pq1 = psum.tile([128, 1024], F32, tag=tg[1], name="pq1", bufs=1)
                    halves = [pq0[:, 0:512], pq0[:, 512:1024],
                              pq1[:, 0:512], pq1[:, 512:1024]]
                    for ki in range(k_tiles):
                        for g in range(NG):
                            mm(halves[g], w_of(ki, m_), rhs_of(ki, g),
                               start=(ki == 0), stop=(ki == k_tiles - 1))
                    a_t = mlp.tile([128, BL], BF16, tag=out_tag, name=out_tag,
                                   bufs=out_bufs)
                    bias = bias_of(m_)
                    if bias is None:
                        nc.scalar.activation(a_t[:, 0:1024], pq0[:], Act.Relu)
                        nc.scalar.activation(a_t[:, 1024:2048], pq1[:], Act.Relu)
                    else:
                        nc.scalar.activation(a_t[:, 0:1024], pq0[:], Act.Relu,
                                             bias=bias)
                        nc.scalar.activation(a_t[:, 1024:2048], pq1[:], Act.Relu,
                                             bias=bias)
                    outs.append(a_t)
                return outs

            def l1_rhs(ki, g):
                if ki == 0:
                    return x0[0:16, GSL[g]]
                return h_cur[ki - 1][:, 512 * (ki - 1) : 512 * ki]

            # layer-1 h rhs: k-chunk k comes from h_cur[g] columns, per group
            def l1_rhs2(ki, g):
                if ki == 0:
                    return x0[0:16, GSL[g]]
                k = ki - 1
                return h_cur[g][:, 512 * k : 512 * (k + 1)]

            def l1_w(ki, m_):
                if ki == 0:
                    return w1x[:, 128 * m_ : 128 * (m_ + 1)]
                return w1h[ki - 1][:, 128 * m_ : 128 * (m_ + 1)]

            a1 = mlp_layer(8, 3, l1_rhs2, l1_w, lambda m_: None, "a1", 8)
            a2 = mlp_layer(8, 8, lambda ki, g: a1[ki][:, GSL[g]],
                           lambda ki, m_: w2[ki][:, 128 * m_ : 128 * (m_ + 1)],
                           lambda m_: b2[:, m_ : m_ + 1], "a2", 8)
            a3 = mlp_layer(4, 8, lambda ki, g: a2[ki][:, GSL[g]],
                           lambda ki, m_: w3[ki][:, 128 * m_ : 128 * (m_ + 1)],
                           lambda m_: b3[:, m_ : m_ + 1], "a3", 4)
            a4 = mlp_layer(2, 4, lambda ki, g: a3[ki][:, GSL[g]],
                           lambda ki, m_: w4[ki][:, 128 * m_ : 128 * (m_ + 1)],
                           lambda m_: b4[:, m_ : m_ + 1], "a4", 2)

            po0 = psum.tile([1, 1024], F32, tag="pR", name="po0", bufs=1)
            po1 = psum.tile([1, 1024], F32, tag="pZ", name="po1", bufs=1)
            phalf = [po0[0:1, 0:512], po0[0:1, 512:1024],
                     po1[0:1, 0:512], po1[0:1, 512:1024]]
            for ki in range(2):
                for g in range(NG):
                    mm(phalf[g], wp[ki][:, 0:1], a4[ki][:, GSL[g]],
                       start=(ki == 0), stop=(ki == 1))
            nc.scalar.activation(oT[0:1, 0:1024], po0[0:1, :], Act.Tanh,
                                 bias=bp[0:1, 0:1])
            nc.scalar.activation(oT[0:1, 1024:2048], po1[0:1, :], Act.Tanh,
                                 bias=bp[0:1, 0:1])

            nc.sync.dma_start(out_d[:], oT[:])

    nc.compile()
    return nc


def _get_nc():
    if "nc" not in _NC_CACHE:
        _NC_CACHE["nc"] = _build_nc()
    return _NC_CACHE["nc"]


def _prep_shared(inputs):
    f4 = np.float32
    bf = ml_dtypes.bfloat16

    def g(name):
        return np.asarray(inputs[name], dtype=f4)

    W_ih, W_hh = g("W_ih"), g("W_hh")
    b_ih, b_hh = g("b_ih"), g("b_hh")
    W1, W2, W3, W4, Wp = g("W1"), g("W2"), g("W3"), g("W4"), g("Wp")
    b1, b2, b3, b4, bp = g("b1"), g("b2"), g("b3"), g("b4"), g("bp")

    # wih_all [128, 1024]: 8 column blocks; block c uses partition strip c%4.
    # blocks 0-3: rz gate chunks (bias = b_ih+b_hh); 4-5: xn chunks
    # (bias = b_ih n-part); 6-7: zero weights + b_hh n-part bias only.
    wih_all = np.zeros((128, 1024), dtype=f4)
    bsum = b_ih + b_hh
    for c in range(4):
        s = 32 * c
        wih_all[s : s + 15, 128 * c : 128 * (c + 1)] = W_ih[128 * c : 128 * (c + 1)].T
        wih_all[s + 15, 128 * c : 128 * (c + 1)] = bsum[128 * c : 128 * (c + 1)]
    for m in range(2):
        c, s = 4 + m, 32 * m
        lo = 512 + 128 * m
        wih_all[s : s + 15, 128 * c : 128 * (c + 1)] = W_ih[lo : lo + 128].T
        wih_all[s + 15, 128 * c : 128 * (c + 1)] = b_ih[lo : lo + 128]
    for m in range(2):
        c, s = 6 + m, 32 * (2 + m)
        lo = 512 + 128 * m
        wih_all[s + 15, 128 * c : 128 * (c + 1)] = b_hh[lo : lo + 128]

    w1x = np.zeros((16, 1024), dtype=f4)
    w1x[0:15] = W1.T[0:15]
    w1x[15] = b1

    shared = {
        "wih_all": wih_all.astype(bf),
        "w_hhT": np.ascontiguousarray(W_hh.T.reshape(2, 128, 768)).astype(bf),
        "w1T_x": w1x.astype(bf),
        "w1T_h": np.ascontiguousarray(W1.T[15:].reshape(2, 128, 1024)).astype(bf),
        "w2T": np.ascontiguousarray(W2.T.reshape(8, 128, 1024)).astype(bf),
        "w3T": np.ascontiguousarray(W3.T.reshape(8, 128, 512)).astype(bf),
        "w4T": np.ascontiguousarray(W4.T.reshape(4, 128, 256)).astype(bf),
        "wpT": np.ascontiguousarray(Wp.T.reshape(2, 128, 1)).astype(bf),
        "b2": np.ascontiguousarray(b2.reshape(8, 128).T),
        "b3": np.ascontiguousarray(b3.reshape(4, 128).T),
        "b4": np.ascontiguousarray(b4.reshape(2, 128).T),
        "bp": bp.reshape(1, 1).astype(f4),
    }
    return shared


def _prep_xT(state_core):
    """state [BL, 20, 15] f32 -> [20, 128, BL] bf16: features replicated at
    partition strips 0/32/64/96; row 15 of each strip is the all-ones bias
    row; rows 16-31 zero."""
    s = state_core.transpose(1, 2, 0)  # [20, 15, BL]
    xp = np.zeros((V, 4, 32, BL), dtype=np.float32)
    xp[:, :, :F, :] = s[:, None, :, :]
    xp[:, :, 15, :] = 1.0
    return xp.reshape(V, 128, BL).astype(ml_dtypes.bfloat16)


def run(inputs, trace=False):
    nc = _get_nc()
    shared = _prep_shared(inputs)
    state = np.asarray(inputs["state"], dtype=np.float32)
    in_maps = []
    for c in range(NCORES):
        m = dict(shared)
        m["xT"] = _prep_xT(state[BL * c : BL * (c + 1)])
        in_maps.append(m)
    res = run_bass_kernel_spmd(nc, in_maps, list(range(NCORES)), trace=trace)
    out = np.concatenate(
        [np.asarray(res.results[c]["out"]).reshape(BL) for c in range(NCORES)]
    )
    return out.reshape(B, 1).astype(np.float32), res


def kernel(**inputs):
    out, _ = run(inputs, trace=False)
    return out
